# revision 1
# baseline (speedup 1.0000x reference)
"""EnergyAttention Trainium2 kernel (8 NeuronCores, head-sharded).

Strategy: shard the 16 heads across 8 cores (2 heads per core). Each core:
  - projects K^T (bf16), K-natural+tens (fp8e4) and Q^T (fp32) for its heads
  - runs 5 energy steps with transposed scores S^T[k, q]; softmax's
    k-reduction rides the grad matmul via an appended tens-column
    (ones column scaled by 10 folds step_size=0.1 into the reciprocal)
  - scores matmul: bf16 (contraction 64 too shallow for fp8 DoubleRow)
  - exp split across engines: ACT does native Exp -> fp8e4; DVE does
    Schraudolph bit-trick exp (x*c1+c2 -> int8, bitcast fp8e4) so both
    engines produce softmax numerators in parallel
  - grad matmul: fp8e4 DoubleRow over k-chunk PAIRS (2x PE throughput)
  - q-update spread over ACT (gt evacuation), DVE (reciprocal) and Pool
    (broadcast/mul/both adds), emission deferred into the next q-block's
    pair loop so no exp queue ever bubbles on the update chain
  - computes its partial output block through its Wo columns (bf16)
Host: transposes/casts inputs, upcasts and sums the 8 bf16 partial outputs.
"""

import numpy as np
import ml_dtypes

BF16 = ml_dtypes.bfloat16

N_CORES = 8
D = 1024
K = 4096
Q = 2048
H = 16
HD = 64
STEPS = 5
STEP_SIZE = 0.1
BETA = 1.0 / np.sqrt(np.float32(HD))  # 1/8

QB = 512

# exp column split: of each chunk's 1024 score columns (2 heads x 512 q),
# ACT takes [0:CA) (native exp) and DVE takes [CA:1024) (Schraudolph)
CA = 540
# Schraudolph fp8e4: p = bitcast_e4m3(int8(s * C1 + C2))  ~= exp(beta*s)
SCH_C1 = float(1.0 / np.log(2.0))        # beta * 8 / ln2 = 1/ln2
SCH_C2 = 56.5 - 0.045 * 8.0              # exp-bias 7*8 + trunc 0.5 - balance

_CACHE = {}


def build_program(d=D, k=K, q=Q, steps=STEPS, n_cores=N_CORES):
    """Build + compile the per-core Bass program. Returns the Bacc object."""
    from contextlib import ExitStack

    import concourse.tile as tile
    from concourse import bacc, mybir

    f32 = mybir.dt.float32
    bf16 = mybir.dt.bfloat16
    fp8 = mybir.dt.float8e4
    i8 = mybir.dt.int8

    ndc = d // 128       # D chunks (contraction for projections)
    nkb = k // 512       # k blocks for K^T projection
    nkc = k // 128       # k chunks for the step loop
    npr = nkc // 2       # k chunk-pairs (DoubleRow grad granularity)
    nqb = q // QB        # q blocks
    beta = float(1.0 / np.sqrt(np.float64(HD)))

    nc = bacc.Bacc("TRN2", target_bir_lowering=False, debug=False,
                   num_devices=n_cores)
    ctxT = nc.dram_tensor("ctxT", [d, k], bf16, kind="ExternalInput").ap()
    tgtT = nc.dram_tensor("tgtT", [d, q], bf16, kind="ExternalInput").ap()
    wk = nc.dram_tensor("wk", [d, 128], bf16, kind="ExternalInput").ap()
    wq = nc.dram_tensor("wq", [d, 128], bf16, kind="ExternalInput").ap()
    woT = nc.dram_tensor("woT", [128, d], bf16, kind="ExternalInput").ap()
    out = nc.dram_tensor("out", [q, d], bf16, kind="ExternalOutput").ap()

    EXP = mybir.ActivationFunctionType.Exp
    DR = mybir.MatmulPerfMode.DoubleRow
    MUL = mybir.AluOpType.mult
    ADD = mybir.AluOpType.add

    with tile.TileContext(nc) as tc, ExitStack() as ctx:
        # ---------------- persistent pools ----------------
        kt_pool = ctx.enter_context(tc.tile_pool(name="kt", bufs=1))
        kon_pool = ctx.enter_context(tc.tile_pool(name="kones", bufs=1))
        qt_pool = ctx.enter_context(tc.tile_pool(name="qt", bufs=2 * nqb))
        qtb_pool = ctx.enter_context(tc.tile_pool(name="qtb", bufs=2 * nqb))
        w_pool = ctx.enter_context(tc.tile_pool(name="w", bufs=1))

        # per-head padded K^T: other head's rows zeroed -> full-128 contraction
        ktp = [kt_pool.tile([128, k], bf16, tag=f"ktp{h}", name=f"ktp{h}")
               for h in range(2)]
        nc.vector.memset(ktp[0][64:128, :], 0.0)
        nc.vector.memset(ktp[1][0:64, :], 0.0)
        # K natural + tens column in fp8e4, pair-major for DoubleRow:
        # [128, pair, i(2), 96]; per chunk col 64 = 10.0 (denominator rider),
        # cols 65-95 padding (DoubleRow ldweights needs multiple-of-32 cols)
        kones = [kon_pool.tile([128, npr, 2, 96], fp8, tag=f"kones{h}",
                               name=f"kones{h}")
                 for h in range(2)]
        wk_sb = w_pool.tile([128, d], bf16, tag="wk")
        wq_sb = w_pool.tile([128, d], bf16, tag="wq")
        wo_sb = w_pool.tile([128, d], bf16, tag="wo")

        # weights ride the ACT DGE ring so their 17 issues don't block the
        # big ctx/tgt streams on the SP ring (wk first: it gates K^T)
        for c in range(ndc):
            cs = slice(c * 128, (c + 1) * 128)
            nc.scalar.dma_start(out=wk_sb[:, cs], in_=wk[cs, :])
        for c in range(ndc):
            cs = slice(c * 128, (c + 1) * 128)
            nc.scalar.dma_start(out=wq_sb[:, cs], in_=wq[cs, :])
        nc.scalar.dma_start(out=wo_sb[:], in_=woT[:])

        # tens+pad columns only (cols 64-95 of each chunk slot): the K-nat
        # evacuations fill cols 0-63, so skip memsetting them
        for h in range(2):
            nc.vector.memset(kones[h][:, :, :, 64:96], 10.0)

        qt_tiles = []
        qtb_tiles = []

        # ---------------- phase A: projections ----------------
        with tc.tile_pool(name="ctxp", bufs=ndc) as ctx_pool, \
             tc.tile_pool(name="tgtp", bufs=ndc) as tgt_pool, \
             tc.tile_pool(name="psA", bufs=2, space="PSUM") as psA, \
             tc.tile_pool(name="psB", bufs=2, space="PSUM") as psB, \
             tc.tile_pool(name="psQ", bufs=2, space="PSUM") as psQ:
            ctx_tiles = [ctx_pool.tile([128, k], bf16, tag="ctx", name=f"ctx{c}")
                         for c in range(ndc)]
            tgt_tiles = [tgt_pool.tile([128, q], bf16, tag="tgt", name=f"tgt{c}")
                         for c in range(ndc)]
            # ctxT arrives in two key-halves: the K^T projection of the
            # first 2048 keys can start after ~8 slice DMAs instead of the
            # full 8MB, while staying under the ~650ns/DMA issue rate
            for khalf in range(2):
                ks = slice(khalf * (k // 2), (khalf + 1) * (k // 2))
                for c in range(ndc):
                    cs = slice(c * 128, (c + 1) * 128)
                    nc.sync.dma_start(out=ctx_tiles[c][:, ks],
                                      in_=ctxT[cs, ks])
            for c in range(ndc):
                cs = slice(c * 128, (c + 1) * 128)
                nc.sync.dma_start(out=tgt_tiles[c][:], in_=tgtT[cs, :])

            # K^T = Wk_pair^T @ context^T  (bf16)
            for kb in range(nkb):
                ks = slice(kb * 512, (kb + 1) * 512)
                pk = psA.tile([128, 512], f32, tag="pk")
                for c in range(ndc):
                    cs = slice(c * 128, (c + 1) * 128)
                    nc.tensor.matmul(out=pk[:], lhsT=wk_sb[:, cs],
                                     rhs=ctx_tiles[c][:, ks],
                                     start=(c == 0), stop=(c == ndc - 1))
                nc.vector.tensor_copy(out=ktp[0][0:64, ks], in_=pk[0:64, :])
                nc.vector.tensor_copy(out=ktp[1][64:128, ks], in_=pk[64:128, :])

            # K natural (both heads side by side), scattered into kones (fp8)
            for kc in range(nkc):
                ks = slice(kc * 128, (kc + 1) * 128)
                pn = psB.tile([128, 128], f32, tag="pn")
                for c in range(ndc):
                    cs = slice(c * 128, (c + 1) * 128)
                    nc.tensor.matmul(out=pn[:], lhsT=ctx_tiles[c][:, ks],
                                     rhs=wk_sb[:, cs],
                                     start=(c == 0), stop=(c == ndc - 1))
                for h in range(2):
                    nc.scalar.copy(
                        out=kones[h][:, kc // 2, kc % 2, 0:64],
                        in_=pn[:, h * 64:(h + 1) * 64])

            # Q^T projection (bf16 inputs, fp32 accumulate)
            for j in range(nqb):
                qs = slice(j * QB, (j + 1) * QB)
                pq = psQ.tile([128, QB], f32, tag="pq")
                for c in range(ndc):
                    cs = slice(c * 128, (c + 1) * 128)
                    nc.tensor.matmul(out=pq[:], lhsT=wq_sb[:, cs],
                                     rhs=tgt_tiles[c][:, qs],
                                     start=(c == 0), stop=(c == ndc - 1))
                q0 = qt_pool.tile([128, QB], f32, tag="qt")
                nc.vector.tensor_copy(out=q0[:], in_=pq[:])
                qb0 = qtb_pool.tile([128, QB], bf16, tag="qtb")
                nc.vector.tensor_copy(out=qb0[:], in_=q0[:])
                qt_tiles.append(q0)
                qtb_tiles.append(qb0)

        # ---------------- phase B: energy steps ----------------
        # Per (step, q-block, k-chunk-pair): scores S^T via zero-padded
        # per-head K^T (bf16), exp on ACT (native) or DVE (Schraudolph) into
        # a shared fp8 p-tile, then one DoubleRow grad matmul per head
        # covering both chunks of the pair. kones col 64 = 10.0 rides the
        # matmul to produce 10*denominator in gt row 64.
        with tc.tile_pool(name="pt", bufs=8) as pt_pool, \
             tc.tile_pool(name="upd", bufs=8) as upd_pool, \
             tc.tile_pool(name="ps_s", bufs=3, space="PSUM") as ps_s, \
             tc.tile_pool(name="ps_g", bufs=2, space="PSUM") as ps_g:
            deferred = []  # queued q-update emitters (see below)
            # grads are emitted one pair late (software pipelining for the
            # in-order PE queue); `pending` carries across q-block boundaries
            pending = None
            for t in range(steps):
                new_qt = []
                new_qtb = []
                for j in range(nqb):
                    qcur = qt_tiles[j]
                    qbcur = qtb_tiles[j]
                    # one accumulator per head, 16 DR matmuls deep
                    gt = [ps_g.tile([96, QB], f32, tag="g", name=f"g{t}_{j}_{i}")
                          for i in range(2)]
                    for pr in range(npr):
                        if pr == 2 and deferred:
                            # emit the previous q-block's update chain here so
                            # no exp queue waits on it at the block boundary
                            for fn in deferred:
                                fn()
                            deferred.clear()
                        p2 = pt_pool.tile([128, 2, 2 * QB], fp8, tag="p2")
                        for i in range(2):
                            kc = 2 * pr + i
                            s = ps_s.tile([128, 2 * QB], f32, tag="s")
                            for h in range(2):
                                nc.tensor.matmul(
                                    out=s[:, h * QB:(h + 1) * QB],
                                    lhsT=ktp[h][:, kc * 128:(kc + 1) * 128],
                                    rhs=qbcur[:, :],
                                    start=True, stop=True)
                            # both engines consume the same scores in parallel
                            nc.scalar.activation(p2[:, i, 0:CA], s[:, 0:CA],
                                                 EXP, scale=beta)
                            nc.vector.tensor_scalar(
                                out=p2[:, i, CA:2 * QB].bitcast(i8),
                                in0=s[:, CA:2 * QB],
                                scalar1=SCH_C1, scalar2=SCH_C2,
                                op0=MUL, op1=ADD)
                        if pending is not None:
                            pgt, gpr, gp2 = pending
                            for h in range(2):
                                nc.tensor.matmul(
                                    out=pgt[h][:], lhsT=kones[h][:, gpr],
                                    rhs=gp2[:, :, h * QB:(h + 1) * QB],
                                    perf_mode=DR, start=(gpr == 0),
                                    stop=(gpr == npr - 1))
                        pending = (gt, pr, p2)
                    # q update: q += (G/10) / (denom/10) * 0.1 == q + 0.1*G/denom
                    # Engine placement keeps every exp queue bubble-free:
                    # ACT evacuates gt (same table as Exp), DVE only does the
                    # tiny reciprocal, and Pool (idle, SBUF-only access) runs
                    # broadcast/mul plus BOTH adds (fp32 q and bf16 q copies).
                    # Emission is deferred into the next q-block's pair loop;
                    # tiles are allocated eagerly so later code can reference
                    # them (instruction deps still guarantee correct order).
                    qn = qt_pool.tile([128, QB], f32, tag="qt")
                    qb_new = qtb_pool.tile([128, QB], bf16, tag="qtb")

                    def make_update(gt=gt, qcur=qcur, qn=qn, qb_new=qb_new):
                        tm = upd_pool.tile([128, QB], f32, tag="tm")
                        for h in range(2):
                            hs = slice(h * 64, (h + 1) * 64)
                            t2 = upd_pool.tile([65, QB], f32, tag="t2")
                            nc.scalar.copy(out=t2[:], in_=gt[h][0:65, :])
                            # reciprocal lands on partition 0: the gpsimd
                            # partition_broadcast only reads correctly from a
                            # partition-0 source on HW
                            r = upd_pool.tile([1, QB], f32, tag="r")
                            nc.vector.reciprocal(out=r[:], in_=t2[64:65, :])
                            rb = upd_pool.tile([64, QB], f32, tag="rb")
                            nc.gpsimd.partition_broadcast(rb[:], r[0:1, :])
                            nc.gpsimd.tensor_mul(out=tm[hs, :],
                                                 in0=t2[0:64, :], in1=rb[:])
                        nc.gpsimd.tensor_add(out=qn[:], in0=qcur[:],
                                             in1=tm[:])
                        nc.gpsimd.tensor_add(out=qb_new[:], in0=qcur[:],
                                             in1=tm[:])

                    deferred.append(make_update)
                    new_qt.append(qn)
                    new_qtb.append(qb_new)
                qt_tiles = new_qt
                qtb_tiles = new_qtb
            # flush the final pair's grads and q-block updates before phase C
            if pending is not None:
                pgt, gpr, gp2 = pending
                for h in range(2):
                    nc.tensor.matmul(
                        out=pgt[h][:], lhsT=kones[h][:, gpr],
                        rhs=gp2[:, :, h * QB:(h + 1) * QB],
                        perf_mode=DR, start=(gpr == 0), stop=(gpr == npr - 1))
                pending = None
            for fn in deferred:
                fn()
            deferred.clear()

        # ---------------- phase C: output projection (bf16) ----------------
        # [128, 1024] PSUM tiles (2 matmuls each); evacuation column-split
        # across ACT+DVE (it, not the matmul, paces this phase); one DMA
        # per 128-row block.
        CC = 528  # ACT's share of the 1024 evacuation columns
        with tc.tile_pool(name="fo", bufs=6) as fo_pool, \
             tc.tile_pool(name="psO", bufs=3, space="PSUM") as psO:
            for qb128 in range(q // 128):
                jt = qtb_tiles[(qb128 * 128) // QB]
                qs = slice((qb128 * 128) % QB, (qb128 * 128) % QB + 128)
                po = psO.tile([128, d], f32, tag="po")
                for half in range(2):
                    ds_ = slice(half * (d // 2), (half + 1) * (d // 2))
                    nc.tensor.matmul(out=po[:, ds_], lhsT=jt[:, qs],
                                     rhs=wo_sb[:, ds_],
                                     start=True, stop=True)
                ot = fo_pool.tile([128, d], bf16, tag="ot")
                nc.scalar.copy(out=ot[:, 0:CC], in_=po[:, 0:CC])
                nc.vector.tensor_copy(out=ot[:, CC:d], in_=po[:, CC:d])
                nc.sync.dma_start(
                    out=out[qb128 * 128:(qb128 + 1) * 128, :],
                    in_=ot[:])

    nc.compile()
    return nc


def _get_program():
    if "nc" not in _CACHE:
        _CACHE["nc"] = build_program()
    return _CACHE["nc"]


def make_in_maps(context, target_init, Wq, Wk, Wo):
    """Host-side sharding/layout prep: one input map per core."""
    ctxT = np.ascontiguousarray(context.T).astype(BF16)        # [D, K]
    tgtT = np.ascontiguousarray(target_init.T).astype(BF16)  # [D, Q]
    in_maps = []
    for c in range(N_CORES):
        h0, h1 = 2 * c, 2 * c + 1
        wk_c = np.concatenate([Wk[h0].T, Wk[h1].T], axis=1)    # [D, 128]
        wq_c = np.concatenate([Wq[h0].T, Wq[h1].T], axis=1)    # [D, 128]
        woT_c = np.ascontiguousarray(Wo[:, 128 * c:128 * (c + 1)].T)  # [128, D]
        in_maps.append({
            "ctxT": ctxT,
            "tgtT": tgtT,
            "wk": np.ascontiguousarray(wk_c).astype(BF16),
            "wq": np.ascontiguousarray(wq_c).astype(BF16),
            "woT": woT_c.astype(BF16),
        })
    return in_maps


def kernel(context, target_init, Wq, Wk, Wo):
    context = np.asarray(context, dtype=np.float32)
    target_init = np.asarray(target_init, dtype=np.float32)
    Wq = np.asarray(Wq, dtype=np.float32)
    Wk = np.asarray(Wk, dtype=np.float32)
    Wo = np.asarray(Wo, dtype=np.float32)

    in_maps = make_in_maps(context, target_init, Wq, Wk, Wo)

    last_err = None
    for _attempt in range(3):
        try:
            results = _run_spmd(in_maps)
            break
        except Exception as e:  # transient axon RESOURCE_EXHAUSTED etc.
            last_err = e
            _CACHE.clear()
    else:
        raise last_err

    acc = np.zeros((Q, D), dtype=np.float32)
    for c in range(N_CORES):
        acc += results[c]["out"].astype(np.float32)
    return acc


def _run_spmd(in_maps):
    """Run the program on cores 0..7. Uses a cached jitted executable with
    device-resident zero buffers; falls back to run_bass_kernel_spmd."""
    nc = _get_program()
    try:
        runner = _CACHE.get("runner")
        if runner is None:
            runner = _SpmdRunner(nc, N_CORES)
            _CACHE["runner"] = runner
        return runner.run(in_maps)
    except Exception:
        _CACHE.pop("runner", None)
        from concourse.bass_utils import run_bass_kernel_spmd
        res = run_bass_kernel_spmd(nc, in_maps, list(range(N_CORES)))
        return res.results


class _SpmdRunner:
    """Persistent jitted shard_map executable (mirrors
    bass2jax.run_bass_via_pjrt's multi-core path, without output donation so
    the executable and zero buffers are reusable across calls)."""

    def __init__(self, nc, n_cores):
        import jax
        from jax.experimental.shard_map import shard_map
        from jax.sharding import Mesh, NamedSharding, PartitionSpec
        import concourse.mybir as mybir
        from concourse.bass2jax import (
            _bass_exec_p, install_neuronx_cc_hook, partition_id_tensor)

        install_neuronx_cc_hook()
        self.jax = jax
        self.n_cores = n_cores
        partition_name = (nc.partition_id_tensor.name
                          if nc.partition_id_tensor else None)
        in_names, out_names, out_avals, zero_outs = [], [], [], []
        for alloc in nc.m.functions[0].allocations:
            if not isinstance(alloc, mybir.MemoryLocationSet):
                continue
            name = alloc.memorylocations[0].name
            if alloc.kind == "ExternalInput":
                if name != partition_name:
                    in_names.append(name)
            elif alloc.kind == "ExternalOutput":
                shape = tuple(alloc.tensor_shape)
                dtype = mybir.dt.np(alloc.dtype)
                out_names.append(name)
                out_avals.append(jax.core.ShapedArray(shape, dtype))
                zero_outs.append(np.zeros(shape, dtype))
        self.in_names = in_names
        self.out_names = out_names
        self.out_avals = out_avals
        all_in_names = in_names + out_names
        if partition_name is not None:
            all_in_names.append(partition_name)

        def _body(*args):
            operands = list(args)
            if partition_name is not None:
                operands.append(partition_id_tensor())
            outs = _bass_exec_p.bind(
                *operands,
                out_avals=tuple(out_avals),
                in_names=tuple(all_in_names),
                out_names=tuple(out_names),
                lowering_input_output_aliases=(),
                sim_require_finite=True,
                sim_require_nnan=True,
                nc=nc,
            )
            return tuple(outs)

        devices = jax.devices()[:n_cores]
        mesh = Mesh(np.asarray(devices), ("core",))
        in_specs = (PartitionSpec("core"),) * (len(in_names) + len(out_names))
        out_specs = (PartitionSpec("core"),) * len(out_names)
        self.fn = jax.jit(
            shard_map(_body, mesh=mesh, in_specs=in_specs,
                      out_specs=out_specs, check_rep=False),
            keep_unused=True,
        )
        self.sharding = NamedSharding(mesh, PartitionSpec("core"))
        self.zeros_placed = [
            jax.device_put(np.concatenate([z] * n_cores, axis=0), self.sharding)
            for z in zero_outs
        ]

    def place(self, in_maps):
        concat = [
            np.concatenate([np.asarray(in_maps[c][n])
                            for c in range(self.n_cores)], axis=0)
            for n in self.in_names
        ]
        return [self.jax.device_put(a, self.sharding) for a in concat]

    def exec_placed(self, placed):
        outs = self.fn(*placed, *self.zeros_placed)
        self.jax.block_until_ready(outs)
        return outs

    def run(self, in_maps):
        outs = self.exec_placed(self.place(in_maps))
        per_core = []
        for c in range(self.n_cores):
            d = {}
            for i, n in enumerate(self.out_names):
                full = np.asarray(outs[i])
                sh = self.out_avals[i].shape
                d[n] = full.reshape(self.n_cores, *sh)[c]
            per_core.append(d)
        return per_core



# revision 7
# speedup vs baseline: 1.2723x; 1.2723x over previous
"""EnergyAttention Trainium2 kernel (8 NeuronCores, head-sharded).

Strategy: shard the 16 heads across 8 cores (2 heads per core). Each core:
  - projects K^T (bf16), K-natural+tens (fp8e4) and Q^T (fp32) for its heads
  - runs 5 energy steps with transposed scores S^T[k, q]; softmax's
    k-reduction rides the grad matmul via an appended tens-column
    (ones column scaled by 10 folds step_size=0.1 into the reciprocal)
  - scores matmul: bf16 (contraction 64 too shallow for fp8 DoubleRow)
  - exp split across engines: ACT does native Exp -> fp8e4; DVE does
    Schraudolph bit-trick exp (x*c1+c2 -> int8, bitcast fp8e4) so both
    engines produce softmax numerators in parallel
  - grad matmul: fp8e4 DoubleRow over k-chunk PAIRS (2x PE throughput)
  - q-update spread over ACT (gt evacuation), DVE (reciprocal) and Pool
    (broadcast/mul/both adds), emission deferred into the next q-block's
    pair loop so no exp queue ever bubbles on the update chain
  - computes its partial output block through its Wo columns (bf16)
Host: transposes/casts inputs, upcasts and sums the 8 bf16 partial outputs.
"""

import numpy as np
import ml_dtypes

BF16 = ml_dtypes.bfloat16

N_CORES = 8
D = 1024
K = 4096
Q = 2048
H = 16
HD = 64
STEPS = 5
STEP_SIZE = 0.1
BETA = 1.0 / np.sqrt(np.float32(HD))  # 1/8

QB = 512

# exp tile alternation: each k-chunk's full [128, 1024] score tile goes to
# ONE engine (ACT native exp, or DVE Schraudolph) -- whole-tile instructions
# amortize the per-instruction PSUM/SBUF access latency. ACT is faster per
# column (1.2 vs 0.96 GHz), so it takes M32 of every 32 chunks.
M32 = 17
# Schraudolph fp8e4: p = bitcast_e4m3(int8(s * C1 + C2))  ~= exp(beta*s)
SCH_C1 = float(1.0 / np.log(2.0))        # beta * 8 / ln2 = 1/ln2
SCH_C2 = 56.5 - 0.045 * 8.0              # exp-bias 7*8 + trunc 0.5 - balance

_CACHE = {}


def build_program(d=D, k=K, q=Q, steps=STEPS, n_cores=N_CORES):
    """Build + compile the per-core Bass program. Returns the Bacc object."""
    from contextlib import ExitStack

    import concourse.tile as tile
    from concourse import bacc, mybir

    f32 = mybir.dt.float32
    bf16 = mybir.dt.bfloat16
    fp8 = mybir.dt.float8e4
    i8 = mybir.dt.int8

    ndc = d // 128       # D chunks (contraction for projections)
    nkb = k // 512       # k blocks for K^T projection
    nkc = k // 128       # k chunks for the step loop
    npr = nkc // 2       # k chunk-pairs (DoubleRow grad granularity)
    nqb = q // QB        # q blocks
    beta = float(1.0 / np.sqrt(np.float64(HD)))

    nc = bacc.Bacc("TRN2", target_bir_lowering=False, debug=False,
                   num_devices=n_cores)
    ctxT = nc.dram_tensor("ctxT", [d, k], bf16, kind="ExternalInput").ap()
    tgtT = nc.dram_tensor("tgtT", [d, q], bf16, kind="ExternalInput").ap()
    wk = nc.dram_tensor("wk", [d, 128], bf16, kind="ExternalInput").ap()
    wq = nc.dram_tensor("wq", [d, 128], bf16, kind="ExternalInput").ap()
    woT = nc.dram_tensor("woT", [128, d], bf16, kind="ExternalInput").ap()
    out = nc.dram_tensor("out", [q, d], bf16, kind="ExternalOutput").ap()

    EXP = mybir.ActivationFunctionType.Exp
    DR = mybir.MatmulPerfMode.DoubleRow
    MUL = mybir.AluOpType.mult
    ADD = mybir.AluOpType.add

    with tile.TileContext(nc) as tc, ExitStack() as ctx:
        # ---------------- persistent pools ----------------
        kt_pool = ctx.enter_context(tc.tile_pool(name="kt", bufs=1))
        kon_pool = ctx.enter_context(tc.tile_pool(name="kones", bufs=1))
        qtb_pool = ctx.enter_context(tc.tile_pool(name="qtb", bufs=2 * nqb))
        w_pool = ctx.enter_context(tc.tile_pool(name="w", bufs=1))

        # K^T both heads stacked: rows 0-63 head0's hd dims, 64-127 head1's.
        # Scores matmuls are row-tiled (tile_position (0,0)/(64,0)) so the two
        # heads' 64-contraction matmuls run CONCURRENTLY in the PE array.
        ktp = kt_pool.tile([128, k], bf16, tag="ktp", name="ktp")
        # K natural + tens column in fp8e4, pair-major for DoubleRow:
        # [128, pair, i(2), 96]; per chunk col 64 = 10.0 (denominator rider),
        # cols 65-95 padding (DoubleRow ldweights needs multiple-of-32 cols)
        kones = [kon_pool.tile([128, npr, 2, 96], fp8, tag=f"kones{h}",
                               name=f"kones{h}")
                 for h in range(2)]
        wk_sb = w_pool.tile([128, d], bf16, tag="wk")
        wq_sb = w_pool.tile([128, d], bf16, tag="wq")
        wo_sb = w_pool.tile([128, d], bf16, tag="wo")

        # weights ride the ACT DGE ring so their 17 issues don't block the
        # big ctx/tgt streams on the SP ring (wk first: it gates K^T)
        for c in range(ndc):
            cs = slice(c * 128, (c + 1) * 128)
            nc.scalar.dma_start(out=wk_sb[:, cs], in_=wk[cs, :])
        for c in range(ndc):
            cs = slice(c * 128, (c + 1) * 128)
            nc.scalar.dma_start(out=wq_sb[:, cs], in_=wq[cs, :])
        nc.scalar.dma_start(out=wo_sb[:], in_=woT[:])

        # tens+pad columns only (cols 64-95 of each chunk slot): the K-nat
        # evacuations fill cols 0-63, so skip memsetting them
        for h in range(2):
            nc.vector.memset(kones[h][:, :, :, 64:96], 10.0)

        qtb_tiles = []

        # ---------------- phase A: projections ----------------
        with tc.tile_pool(name="ctxp", bufs=ndc) as ctx_pool, \
             tc.tile_pool(name="tgtp", bufs=ndc) as tgt_pool, \
             tc.tile_pool(name="psA", bufs=2, space="PSUM") as psA, \
             tc.tile_pool(name="psB", bufs=2, space="PSUM") as psB, \
             tc.tile_pool(name="psQ", bufs=2, space="PSUM") as psQ:
            ctx_tiles = [ctx_pool.tile([128, k], bf16, tag="ctx", name=f"ctx{c}")
                         for c in range(ndc)]
            tgt_tiles = [tgt_pool.tile([128, q], bf16, tag="tgt", name=f"tgt{c}")
                         for c in range(ndc)]
            # ctxT arrives in two key-halves: the K^T projection of the
            # first 2048 keys can start after ~8 slice DMAs instead of the
            # full 8MB, while staying under the ~650ns/DMA issue rate
            for khalf in range(2):
                ks = slice(khalf * (k // 2), (khalf + 1) * (k // 2))
                for c in range(ndc):
                    cs = slice(c * 128, (c + 1) * 128)
                    nc.sync.dma_start(out=ctx_tiles[c][:, ks],
                                      in_=ctxT[cs, ks])
            for c in range(ndc):
                cs = slice(c * 128, (c + 1) * 128)
                nc.sync.dma_start(out=tgt_tiles[c][:], in_=tgtT[cs, :])

            # K^T = Wk_pair^T @ context^T  (bf16)
            for kb in range(nkb):
                ks = slice(kb * 512, (kb + 1) * 512)
                pk = psA.tile([128, 512], f32, tag="pk")
                for c in range(ndc):
                    cs = slice(c * 128, (c + 1) * 128)
                    nc.tensor.matmul(out=pk[:], lhsT=wk_sb[:, cs],
                                     rhs=ctx_tiles[c][:, ks],
                                     start=(c == 0), stop=(c == ndc - 1))
                nc.vector.tensor_copy(out=ktp[:, ks], in_=pk[:])

            # K natural (both heads side by side), scattered into kones (fp8)
            for kc in range(nkc):
                ks = slice(kc * 128, (kc + 1) * 128)
                pn = psB.tile([128, 128], f32, tag="pn")
                for c in range(ndc):
                    cs = slice(c * 128, (c + 1) * 128)
                    nc.tensor.matmul(out=pn[:], lhsT=ctx_tiles[c][:, ks],
                                     rhs=wk_sb[:, cs],
                                     start=(c == 0), stop=(c == ndc - 1))
                for h in range(2):
                    nc.scalar.copy(
                        out=kones[h][:, kc // 2, kc % 2, 0:64],
                        in_=pn[:, h * 64:(h + 1) * 64])

            # Q^T projection (bf16 inputs, fp32 accumulate, bf16 kept)
            for j in range(nqb):
                qs = slice(j * QB, (j + 1) * QB)
                pq = psQ.tile([128, QB], f32, tag="pq")
                for c in range(ndc):
                    cs = slice(c * 128, (c + 1) * 128)
                    nc.tensor.matmul(out=pq[:], lhsT=wq_sb[:, cs],
                                     rhs=tgt_tiles[c][:, qs],
                                     start=(c == 0), stop=(c == ndc - 1))
                qb0 = qtb_pool.tile([128, QB], bf16, tag="qtb")
                nc.vector.tensor_copy(out=qb0[:], in_=pq[:])
                qtb_tiles.append(qb0)

        # ---------------- phase B: energy steps ----------------
        # Per (step, q-block, k-chunk-pair): scores S^T via row-tiled per-head
        # matmuls (both heads concurrent in the PE array), exp of each chunk's
        # WHOLE [128, 1024] score tile on ONE engine (ACT native exp or DVE
        # Schraudolph, alternating ~17:15) into a shared fp8 p-tile, then one
        # DoubleRow grad matmul per head covering both chunks of the pair.
        # kones col 64 = 10.0 rides the matmul to produce 10*denominator in
        # gt row 64.
        exp_cnt = 0  # Bresenham counter for the ACT:DVE tile alternation
        with tc.tile_pool(name="pt", bufs=8) as pt_pool, \
             tc.tile_pool(name="upd", bufs=8) as upd_pool, \
             tc.tile_pool(name="ps_s", bufs=3, space="PSUM") as ps_s, \
             tc.tile_pool(name="ps_g", bufs=2, space="PSUM") as ps_g:
            deferred = []  # queued q-update emitters (see below)
            # grads are emitted one pair late (software pipelining for the
            # in-order PE queue); `pending` carries across q-block boundaries
            pending = None
            for t in range(steps):
                new_qtb = []
                for j in range(nqb):
                    qbcur = qtb_tiles[j]
                    # one accumulator per head, 16 DR matmuls deep
                    gt = [ps_g.tile([96, QB], f32, tag="g", name=f"g{t}_{j}_{i}")
                          for i in range(2)]
                    for pr in range(npr):
                        if pr == 2 and deferred:
                            # emit the previous q-block's update chain here so
                            # no exp queue waits on it at the block boundary
                            for fn in deferred:
                                fn()
                            deferred.clear()
                        p2 = pt_pool.tile([128, 2, 2 * QB], fp8, tag="p2")
                        for i in range(2):
                            kc = 2 * pr + i
                            s = ps_s.tile([128, 2 * QB], f32, tag="s")
                            for h in range(2):
                                hs = slice(h * 64, (h + 1) * 64)
                                nc.tensor.matmul(
                                    out=s[:, h * QB:(h + 1) * QB],
                                    lhsT=ktp[hs, kc * 128:(kc + 1) * 128],
                                    rhs=qbcur[hs, :],
                                    start=True, stop=True)
                            # whole-tile exp on one engine, alternating
                            on_act = ((exp_cnt * M32) // 32
                                      != ((exp_cnt + 1) * M32) // 32)
                            exp_cnt += 1
                            if on_act:
                                nc.scalar.activation(p2[:, i, :], s[:],
                                                     EXP, scale=beta)
                            else:
                                nc.vector.tensor_scalar(
                                    out=p2[:, i, :].bitcast(i8),
                                    in0=s[:],
                                    scalar1=SCH_C1, scalar2=SCH_C2,
                                    op0=MUL, op1=ADD)
                        if pending is not None:
                            pgt, gpr, gp2 = pending
                            for h in range(2):
                                nc.tensor.matmul(
                                    out=pgt[h][:], lhsT=kones[h][:, gpr],
                                    rhs=gp2[:, :, h * QB:(h + 1) * QB],
                                    perf_mode=DR, start=(gpr == 0),
                                    stop=(gpr == npr - 1))
                        pending = (gt, pr, p2)
                    # q update: q += (G/10) / (denom/10) * 0.1 == q + 0.1*G/denom
                    # Engine placement keeps every exp queue bubble-free:
                    # ACT evacuates gt (same table as Exp), DVE only does the
                    # tiny reciprocal, and Pool (idle, SBUF-only access) runs
                    # broadcast/mul plus the bf16 q add (q lives in bf16 only;
                    # the fp32->bf16 rounding across 5 updates stays well
                    # inside the error budget).
                    # Emission is deferred into the next q-block's pair loop;
                    # tiles are allocated eagerly so later code can reference
                    # them (instruction deps still guarantee correct order).
                    qb_new = qtb_pool.tile([128, QB], bf16, tag="qtb")

                    def make_update(gt=gt, qbcur=qbcur, qb_new=qb_new):
                        tm = upd_pool.tile([128, QB], f32, tag="tm")
                        for h in range(2):
                            hs = slice(h * 64, (h + 1) * 64)
                            t2 = upd_pool.tile([65, QB], f32, tag="t2")
                            nc.scalar.copy(out=t2[:], in_=gt[h][0:65, :])
                            # reciprocal lands on partition 0: the gpsimd
                            # partition_broadcast only reads correctly from a
                            # partition-0 source on HW
                            r = upd_pool.tile([1, QB], f32, tag="r")
                            nc.vector.reciprocal(out=r[:], in_=t2[64:65, :])
                            rb = upd_pool.tile([64, QB], f32, tag="rb")
                            nc.gpsimd.partition_broadcast(rb[:], r[0:1, :])
                            nc.gpsimd.tensor_mul(out=tm[hs, :],
                                                 in0=t2[0:64, :], in1=rb[:])
                        nc.gpsimd.tensor_add(out=qb_new[:], in0=qbcur[:],
                                             in1=tm[:])

                    deferred.append(make_update)
                    new_qtb.append(qb_new)
                qtb_tiles = new_qtb
            # flush the final pair's grads and q-block updates before phase C
            if pending is not None:
                pgt, gpr, gp2 = pending
                for h in range(2):
                    nc.tensor.matmul(
                        out=pgt[h][:], lhsT=kones[h][:, gpr],
                        rhs=gp2[:, :, h * QB:(h + 1) * QB],
                        perf_mode=DR, start=(gpr == 0), stop=(gpr == npr - 1))
                pending = None
            for fn in deferred:
                fn()
            deferred.clear()

        # ---------------- phase C: output projection (bf16) ----------------
        # [128, 1024] PSUM tiles (2 matmuls each); evacuation column-split
        # across ACT+DVE (it, not the matmul, paces this phase); one DMA
        # per 128-row block.
        CC = 528  # ACT's share of the 1024 evacuation columns
        with tc.tile_pool(name="fo", bufs=6) as fo_pool, \
             tc.tile_pool(name="psO", bufs=3, space="PSUM") as psO:
            for qb128 in range(q // 128):
                jt = qtb_tiles[(qb128 * 128) // QB]
                qs = slice((qb128 * 128) % QB, (qb128 * 128) % QB + 128)
                po = psO.tile([128, d], f32, tag="po")
                for half in range(2):
                    ds_ = slice(half * (d // 2), (half + 1) * (d // 2))
                    nc.tensor.matmul(out=po[:, ds_], lhsT=jt[:, qs],
                                     rhs=wo_sb[:, ds_],
                                     start=True, stop=True)
                ot = fo_pool.tile([128, d], bf16, tag="ot")
                nc.scalar.copy(out=ot[:, 0:CC], in_=po[:, 0:CC])
                nc.vector.tensor_copy(out=ot[:, CC:d], in_=po[:, CC:d])
                nc.sync.dma_start(
                    out=out[qb128 * 128:(qb128 + 1) * 128, :],
                    in_=ot[:])

    nc.compile()
    return nc


def _get_program():
    if "nc" not in _CACHE:
        _CACHE["nc"] = build_program()
    return _CACHE["nc"]


def make_in_maps(context, target_init, Wq, Wk, Wo):
    """Host-side sharding/layout prep: one input map per core."""
    ctxT = np.ascontiguousarray(context.T).astype(BF16)        # [D, K]
    tgtT = np.ascontiguousarray(target_init.T).astype(BF16)  # [D, Q]
    in_maps = []
    for c in range(N_CORES):
        h0, h1 = 2 * c, 2 * c + 1
        wk_c = np.concatenate([Wk[h0].T, Wk[h1].T], axis=1)    # [D, 128]
        wq_c = np.concatenate([Wq[h0].T, Wq[h1].T], axis=1)    # [D, 128]
        woT_c = np.ascontiguousarray(Wo[:, 128 * c:128 * (c + 1)].T)  # [128, D]
        in_maps.append({
            "ctxT": ctxT,
            "tgtT": tgtT,
            "wk": np.ascontiguousarray(wk_c).astype(BF16),
            "wq": np.ascontiguousarray(wq_c).astype(BF16),
            "woT": woT_c.astype(BF16),
        })
    return in_maps


def kernel(context, target_init, Wq, Wk, Wo):
    context = np.asarray(context, dtype=np.float32)
    target_init = np.asarray(target_init, dtype=np.float32)
    Wq = np.asarray(Wq, dtype=np.float32)
    Wk = np.asarray(Wk, dtype=np.float32)
    Wo = np.asarray(Wo, dtype=np.float32)

    in_maps = make_in_maps(context, target_init, Wq, Wk, Wo)

    last_err = None
    for _attempt in range(3):
        try:
            results = _run_spmd(in_maps)
            break
        except Exception as e:  # transient axon RESOURCE_EXHAUSTED etc.
            last_err = e
            _CACHE.clear()
    else:
        raise last_err

    acc = np.zeros((Q, D), dtype=np.float32)
    for c in range(N_CORES):
        acc += results[c]["out"].astype(np.float32)
    return acc


def _run_spmd(in_maps):
    """Run the program on cores 0..7. Uses a cached jitted executable with
    device-resident zero buffers; falls back to run_bass_kernel_spmd."""
    nc = _get_program()
    try:
        runner = _CACHE.get("runner")
        if runner is None:
            runner = _SpmdRunner(nc, N_CORES)
            _CACHE["runner"] = runner
        return runner.run(in_maps)
    except Exception:
        _CACHE.pop("runner", None)
        from concourse.bass_utils import run_bass_kernel_spmd
        res = run_bass_kernel_spmd(nc, in_maps, list(range(N_CORES)))
        return res.results


class _SpmdRunner:
    """Persistent jitted shard_map executable (mirrors
    bass2jax.run_bass_via_pjrt's multi-core path, without output donation so
    the executable and zero buffers are reusable across calls)."""

    def __init__(self, nc, n_cores):
        import jax
        from jax.experimental.shard_map import shard_map
        from jax.sharding import Mesh, NamedSharding, PartitionSpec
        import concourse.mybir as mybir
        from concourse.bass2jax import (
            _bass_exec_p, install_neuronx_cc_hook, partition_id_tensor)

        install_neuronx_cc_hook()
        self.jax = jax
        self.n_cores = n_cores
        partition_name = (nc.partition_id_tensor.name
                          if nc.partition_id_tensor else None)
        in_names, out_names, out_avals, zero_outs = [], [], [], []
        for alloc in nc.m.functions[0].allocations:
            if not isinstance(alloc, mybir.MemoryLocationSet):
                continue
            name = alloc.memorylocations[0].name
            if alloc.kind == "ExternalInput":
                if name != partition_name:
                    in_names.append(name)
            elif alloc.kind == "ExternalOutput":
                shape = tuple(alloc.tensor_shape)
                dtype = mybir.dt.np(alloc.dtype)
                out_names.append(name)
                out_avals.append(jax.core.ShapedArray(shape, dtype))
                zero_outs.append(np.zeros(shape, dtype))
        self.in_names = in_names
        self.out_names = out_names
        self.out_avals = out_avals
        all_in_names = in_names + out_names
        if partition_name is not None:
            all_in_names.append(partition_name)

        def _body(*args):
            operands = list(args)
            if partition_name is not None:
                operands.append(partition_id_tensor())
            outs = _bass_exec_p.bind(
                *operands,
                out_avals=tuple(out_avals),
                in_names=tuple(all_in_names),
                out_names=tuple(out_names),
                lowering_input_output_aliases=(),
                sim_require_finite=True,
                sim_require_nnan=True,
                nc=nc,
            )
            return tuple(outs)

        devices = jax.devices()[:n_cores]
        mesh = Mesh(np.asarray(devices), ("core",))
        in_specs = (PartitionSpec("core"),) * (len(in_names) + len(out_names))
        out_specs = (PartitionSpec("core"),) * len(out_names)
        self.fn = jax.jit(
            shard_map(_body, mesh=mesh, in_specs=in_specs,
                      out_specs=out_specs, check_rep=False),
            keep_unused=True,
        )
        self.sharding = NamedSharding(mesh, PartitionSpec("core"))
        self.zeros_placed = [
            jax.device_put(np.concatenate([z] * n_cores, axis=0), self.sharding)
            for z in zero_outs
        ]

    def place(self, in_maps):
        concat = [
            np.concatenate([np.asarray(in_maps[c][n])
                            for c in range(self.n_cores)], axis=0)
            for n in self.in_names
        ]
        return [self.jax.device_put(a, self.sharding) for a in concat]

    def exec_placed(self, placed):
        outs = self.fn(*placed, *self.zeros_placed)
        self.jax.block_until_ready(outs)
        return outs

    def run(self, in_maps):
        outs = self.exec_placed(self.place(in_maps))
        per_core = []
        for c in range(self.n_cores):
            d = {}
            for i, n in enumerate(self.out_names):
                full = np.asarray(outs[i])
                sh = self.out_avals[i].shape
                d[n] = full.reshape(self.n_cores, *sh)[c]
            per_core.append(d)
        return per_core

